# revision 25
# baseline (speedup 1.0000x reference)
"""BlurAwareSwinAttentionBlock kernel for 8 Trainium2 NeuronCores.

Data-parallel over batch B=8 (one batch element per core). Each core processes
16 window-row stripes of 1024 tokens in window-major token order
(t = wx*64 + dy*8 + dx). Feature-major activation layouts [feat, tokens] keep
every matmul contraction on the partition axis. LayerNorm token-means are
removed with rank-1 matmul updates in PSUM; the softmax denominator cancels
inside LayerNorm (valid because proj_b == 0, which the spec guarantees).

dtypes: QKV runs float32r (TF32-like, DMA-produced operands only); attention,
proj, FF1, FF2 run fp16 with fp32 PSUM accumulation; residual stream is fp32.
"""
import os
import sys
from contextlib import ExitStack
from types import SimpleNamespace

import numpy as np

sys.path.insert(0, "/opt/trn_rl_repo")

import concourse.bacc as bacc
import concourse.tile as tile
from concourse import mybir
from concourse.bass_utils import run_bass_kernel_spmd

AF = mybir.ActivationFunctionType
ALU = mybir.AluOpType
DT = mybir.dt

B, C, H, W = 8, 256, 128, 128
WS = 8
NUM_HEADS = 8
HD = C // NUM_HEADS          # 32
T = WS * WS                  # 64
FF = 1024
EPS = 1e-5
BLUR_STRENGTH = 1.0
SCALE = C ** (-0.5)

NW_X = W // WS               # 16 windows per stripe
N_STRIPES = H // WS          # 16
TOK = WS * W                 # 1024 tokens per stripe
NPAIR = NW_X // 2            # 8 window pairs per stripe

F32R = DT.float32r
F16 = DT.float16
F32 = DT.float32

_CACHED = {}


def _bilinear_resize_x4(blur):
    """jax.image.resize(blur, (B,1,H,W), 'bilinear') in numpy (half-pixel
    centers, clamped edges)."""
    b, _, hs, ws_ = blur.shape
    out_h, out_w = hs * 4, ws_ * 4

    def axis_weights(n_out, n_in):
        src = (np.arange(n_out) + 0.5) * (n_in / n_out) - 0.5
        i0 = np.floor(src).astype(np.int64)
        frac = (src - i0).astype(np.float32)
        i1 = np.clip(i0 + 1, 0, n_in - 1)
        i0 = np.clip(i0, 0, n_in - 1)
        return i0, i1, frac

    y0, y1, fy = axis_weights(out_h, hs)
    x0, x1, fx = axis_weights(out_w, ws_)
    img = blur[:, 0]
    top = img[:, y0][:, :, x0] * (1 - fx) + img[:, y0][:, :, x1] * fx
    bot = img[:, y1][:, :, x0] * (1 - fx) + img[:, y1][:, :, x1] * fx
    out = top * (1 - fy)[None, :, None] + bot * fy[None, :, None]
    return out[:, None]


def _qkv(nc, E, s, x_r, x16_w):
    """Q, K, V fp16 off window-major x; contiguous psum drains."""
    q_s = E.qkvp.tile([128, 2, TOK], F16, name=f"q_s{s}", tag="q_s")
    k_s = E.qkvp.tile([128, 2, TOK], F16, name=f"k_s{s}", tag="k_s")
    for half in range(2):
        tok = slice(half * 512, (half + 1) * 512)
        for mc in range(2):
            msl = slice(mc * 128, (mc + 1) * 128)
            pq = E.ps_lin.tile([128, 512], F32, name=f"pq{s}_{half}_{mc}", tag="plin")
            for kc in range(2):
                nc.tensor.matmul(pq, E.wq_s[:, kc, msl], x16_w[:, kc, tok],
                                 start=(kc == 0), stop=(kc == 1))
            nc.scalar.activation(out=q_s[:, mc, tok], in_=pq, func=AF.Copy)
            pk = E.ps_lin.tile([128, 512], F32, name=f"pk{s}_{half}_{mc}", tag="plin")
            for kc in range(2):
                nc.tensor.matmul(pk, E.wk_s[:, kc, msl], x16_w[:, kc, tok],
                                 start=(kc == 0), stop=(kc == 1))
            nc.vector.tensor_copy(out=k_s[:, mc, tok], in_=pk)
    v_s = E.qkvp.tile([128, NPAIR, C], F16, name=f"v_s{s}", tag="v_s")
    for p in range(NPAIR):
        pv = E.ps_lin.tile([128, 512], F32, name=f"pv{s}_{p}", tag="plin")
        for kc in range(2):
            nc.tensor.matmul(pv[:, 0:C], x16_w[:, kc, p * 128:(p + 1) * 128],
                             E.wv_s[:, kc, :], start=(kc == 0), stop=(kc == 1))
        nc.vector.tensor_copy(out=v_s[:, p, :], in_=pv[:, 0:C])
    return q_s, k_s, v_s


def _attention(nc, E, s, q_s, k_s, v_s):
    av_s = E.avp.tile([128, 2, TOK], F16, name=f"av_s{s}", tag="av_s")
    psc = E.ps_sc.tile([128, 4, 512], F32, name=f"psc{s}", tag="psc")
    pav = E.ps_av.tile([128, 512], F32, name=f"pav{s}", tag="pav")
    for g in range(NPAIR // 4):
        es_g = []
        pden4 = E.ps_row.tile([128, 512], F32, name=f"pden{s}_{g}", tag="prow")
        for q in range(4):
            p = g * 4 + q
            for c in range(2):
                for hh in range(4):
                    ksl = slice(32 * hh, 32 * hh + 32)
                    for wn, colb in ((0, 0), (1, 64)):
                        wt = slice((2 * p + wn) * T, (2 * p + wn + 1) * T)
                        nc.tensor.matmul(
                            psc[colb:colb + 64, hh, c * 64:(c + 1) * 64],
                            k_s[ksl, c, wt], q_s[ksl, c, wt],
                            start=True, stop=True, tile_position=(32 * hh, colb))
            e_s = E.ep.tile([128, 8, T], F16, name=f"e_s{s}_{p}", tag="e_s")
            nc.scalar.activation(
                out=e_s.rearrange("p (c hh) i -> p hh c i", c=2),
                in_=psc[:, :, 0:128].rearrange("p hh (c i) -> p hh c i", c=2),
                func=AF.Exp, scale=E.blur_s[:, s, p:p + 1])
            es_g.append(e_s)
            # D[win, (h, i)] = sum_j E^T  -> psum rows 32q, 32q+1
            nc.tensor.matmul(pden4[32 * q:32 * q + 32, :], E.ones2,
                             e_s.rearrange("p h i -> p (h i)"),
                             start=True, stop=True, tile_position=(0, 32 * q))
        # batched reciprocal of the 8 denominator rows, then stage to DRAM
        ldn = E.dbp.tile([128, 512], F32, name=f"ldn{s}_{g}", tag="rscr")
        nc.scalar.activation(out=ldn, in_=pden4, func=AF.Ln)
        rd_s = E.dbp.tile([128, 512], F16, name=f"rd_s{s}_{g}", tag="rd_s")
        nc.scalar.activation(out=rd_s, in_=ldn, func=AF.Exp, scale=-1.0)
        for q in range(4):
            p = g * 4 + q
            e_s = es_g[q]
            prb = E.ps_row.tile([128, 512], F32, name=f"prb{s}_{p}", tag="prow")
            nc.tensor.matmul(prb, E.sel_s[32 * q:32 * q + 2, :],
                             rd_s[32 * q:32 * q + 2, :], start=True, stop=True,
                             tile_position=(32 * q, 0))
            e2_s = E.ep.tile([128, 8, T], F16, name=f"e2_s{s}_{p}", tag="e2_s")
            nc.vector.tensor_tensor(
                out=e2_s, in0=prb.rearrange("p (h i) -> p h i", h=8),
                in1=e_s, op=ALU.mult)
            for wn in range(2):
                jsl = slice(wn * 64, wn * 64 + 64)
                for c in range(2):
                    for hh in range(4):
                        h = c * 4 + hh
                        nc.tensor.matmul(
                            pav[32 * hh:32 * hh + 32, c * 64:(c + 1) * 64],
                            v_s[jsl, p, h * HD:(h + 1) * HD], e2_s[jsl, h, :],
                            start=True, stop=True, tile_position=(wn * 64, 32 * hh))
                nc.vector.tensor_copy(
                    out=av_s[:, :, p * 128 + wn * 64: p * 128 + wn * 64 + 64],
                    in_=pav[:, 0:128].rearrange("p (c i) -> p c i", c=2))
    return av_s


def _ln_tail(nc, E, s, half, ln, psums, res, outs, g, b):
    """yp=(y-mu) -> Square -> column-sum -> Ln/Exp rsqrt -> ones-MM
    broadcast into PSUM -> (yp*A)*g + b + res."""
    pvar = E.ps_row.tile([2, 512], F32, name=f"pvar{ln}{s}_{half}", tag="prow")
    yps = []
    for mc in range(2):
        yp = E.sqp.tile([128, 512], F16, name=f"yp{ln}{s}_{half}_{mc}",
                        tag=f"yp{ln}")
        nc.scalar.activation(out=yp, in_=psums[mc], func=AF.Copy)
        yps.append(yp)
        sq = E.sqp.tile([128, 512], F16, name=f"sq{ln}{s}_{half}_{mc}",
                        tag=f"sq{ln}")
        nc.scalar.activation(out=sq, in_=yp, func=AF.Square)
        nc.tensor.matmul(pvar[0:1, :], E.ones_col, sq,
                         start=(mc == 0), stop=(mc == 1))
    srow = E.rowp.tile([1, 512], F32, name=f"srow{ln}{s}_{half}", tag=f"srow{ln}")
    nc.scalar.activation(out=srow, in_=pvar[0:1, :], func=AF.Ln,
                         scale=1.0 / C, bias=E.eps_s[0:1, :])
    arow = E.rowp.tile([1, 512], F16, name=f"arow{ln}{s}_{half}", tag=f"arow{ln}")
    nc.scalar.activation(out=arow, in_=srow, func=AF.Exp, scale=-0.5)
    pab = E.ps_row.tile([128, 512], F32, name=f"pab{ln}{s}_{half}", tag="prow")
    nc.tensor.matmul(pab, E.ones_row, arow, start=True, stop=True)
    for mc in range(2):
        wt_ = E.wtp.tile([128, 512], F32, name=f"wt{ln}{s}_{half}_{mc}", tag=f"wt{ln}")
        nc.vector.tensor_mul(out=wt_, in0=pab, in1=yps[mc])
        nc.vector.affine_then_add(
            out=outs[mc], in0=wt_, in1=res[mc],
            scale=g[:, mc:mc + 1], bias=b[:, mc:mc + 1])


def _proj_ln1(nc, E, s, av_s, x32_w):
    """x1 = x + LN1(proj(av)); all operands window-major contiguous."""
    x1_s = E.x1p.tile([128, 2, TOK], F32, name=f"x1_s{s}", tag="x1_s")
    for half in range(2):
        tok = slice(half * 512, (half + 1) * 512)
        pmu = E.ps_row.tile([2, 512], F32, name=f"pmu{s}_{half}", tag="prow")
        for kc in range(2):
            nc.tensor.matmul(pmu[0:1, :], E.wsp_s[:, kc, :], av_s[:, kc, tok],
                             start=(kc == 0), stop=(kc == 1))
        negmu = E.rowp.tile([1, 512], F16, name=f"negmu{s}_{half}", tag="negmu")
        nc.vector.tensor_scalar(out=negmu, in0=pmu[0:1, :],
                                scalar1=-1.0 / C, scalar2=None, op0=ALU.mult)
        pp = []
        for mc in range(2):
            t_ = E.ps_lin.tile([128, 512], F32, name=f"pp{s}_{half}_{mc}", tag="plin")
            pp.append(t_)
            for kc in range(2):
                nc.tensor.matmul(t_, E.wp_s[:, kc, mc * 128:(mc + 1) * 128],
                                 av_s[:, kc, tok], start=(kc == 0), stop=False)
            nc.tensor.matmul(t_, E.ones_row, negmu, start=False, stop=True)
        _ln_tail(nc, E, s, half, 1, pp,
                 [x32_w[:, mc, tok] for mc in range(2)],
                 [x1_s[:, mc, tok] for mc in range(2)],
                 E.n1g_s, E.n1b_s)
    x1h = E.x1p.tile([128, 2, TOK], F16, name=f"x1h{s}", tag="x1h")
    nc.gpsimd.tensor_copy(out=x1h, in_=x1_s)
    return x1_s, x1h


def _ffn_ln2(nc, E, s, x1_s, x1h, dbg=None):
    x2_w = E.x2p.tile([128, 2, TOK], F32, name=f"x2_w{s}", tag="x2_w")
    for half in range(2):
        tok = slice(half * 512, (half + 1) * 512)
        h_s = E.hp.tile([128, 8, 512], F16, name=f"h_s{s}_{half}", tag="h_s")
        for mc in range(8):
            ph = E.ps_lin.tile([128, 512], F32, name=f"ph{s}_{half}_{mc}", tag="plin")
            for kc in range(2):
                nc.tensor.matmul(ph, E.w1_s[:, kc, mc * 128:(mc + 1) * 128],
                                 x1h[:, kc, tok], start=(kc == 0), stop=(kc == 1))
            if mc % 2 == 0:
                nc.scalar.activation(out=h_s[:, mc, :], in_=ph, func=AF.Relu)
            else:
                nc.vector.tensor_scalar(out=h_s[:, mc, :], in0=ph,
                                        scalar1=0.0, scalar2=None, op0=ALU.max)
        pmu2 = E.ps_row.tile([2, 512], F32, name=f"pmu2{s}_{half}", tag="prow")
        for kc in range(8):
            nc.tensor.matmul(pmu2[0:1, :], E.ws2_s[:, kc, :], h_s[:, kc, :],
                             start=(kc == 0), stop=(kc == 7))
        negmu2 = E.rowp.tile([1, 512], F16, name=f"negmu2{s}_{half}", tag="negmu2")
        nc.vector.tensor_scalar(out=negmu2, in0=pmu2[0:1, :],
                                scalar1=-1.0 / C, scalar2=None, op0=ALU.mult)
        pz = []
        for mc in range(2):
            t_ = E.ps_lin.tile([128, 512], F32, name=f"pz{s}_{half}_{mc}", tag="plin")
            pz.append(t_)
            for kc in range(8):
                nc.tensor.matmul(t_, E.w2_s[:, kc, mc * 128:(mc + 1) * 128],
                                 h_s[:, kc, :], start=(kc == 0), stop=False)
            nc.tensor.matmul(t_, E.ones_row, negmu2, start=False, stop=True)
        if dbg is not None and s == 0 and half == 0:
            nc.sync.dma_start(out=dbg["h"][:, :, :], in_=h_s)
        _ln_tail(nc, E, s, half, 2, pz,
                 [x1_s[:, mc, tok] for mc in range(2)],
                 [x2_w[:, mc, tok] for mc in range(2)],
                 E.n2g_s, E.n2b_s)
    return x2_w


def _stripe(nc, E, s, x_d, out_d, dbg=None):
    # raster load: one contiguous [128, 1024] DMA per chunk
    x_r = E.xin.tile([128, 2, TOK], F32, name=f"x_r{s}", tag="x_r")
    for kc in range(2):
        nc.sync.dma_start(
            out=x_r[:, kc, :],
            in_=x_d[kc, :, s * WS:(s + 1) * WS, :]
                .rearrange("c y x -> c (y x)"))
    # window-major copies via GPSIMD gather: fp16 (V matmul) + fp32 (residual)
    x16_w = E.xin.tile([128, 2, TOK], F16, name=f"x16_w{s}", tag="x16_w")
    x32_w = E.xin.tile([128, 2, TOK], F32, name=f"x32_w{s}", tag="x32_w")
    for kc in range(2):
        src = (x_r[:, kc, :]
               .rearrange("p (y w d) -> p w y d", y=WS, w=NW_X))
        nc.gpsimd.tensor_copy(
            out=x16_w[:, kc, :].rearrange("p (w y d) -> p w y d", w=NW_X, y=WS),
            in_=src)
        nc.gpsimd.tensor_copy(
            out=x32_w[:, kc, :].rearrange("p (w y d) -> p w y d", w=NW_X, y=WS),
            in_=src)
    q_s, k_s, v_s = _qkv(nc, E, s, x_r, x16_w)
    av_s = _attention(nc, E, s, q_s, k_s, v_s)
    x1_s, x1h = _proj_ln1(nc, E, s, av_s, x32_w)
    x2_w = _ffn_ln2(nc, E, s, x1_s, x1h, dbg)
    if dbg is not None and s == 0:
        nc.sync.dma_start(out=dbg["x1h"][:, :, :], in_=x1h)
        for nm, t in (("q", q_s), ("k", k_s), ("av", av_s), ("x1", x1_s)):
            nc.sync.dma_start(out=dbg[nm][:, :, :], in_=t[:, :, :] if nm != "v" else t)
        nc.sync.dma_start(out=dbg["v"][:, :, :], in_=v_s)
    # scatter store: one DMA per (chunk, y-row), alternating sync/scalar
    # queues so stores don't serialize the next stripe's x_r load on sync
    x2v = x2_w.rearrange("p k (w y d) -> p k w y d", w=NW_X, y=WS)
    for kc in range(2):
        for y in range(WS):
            eng = nc.sync if (kc * WS + y) % 2 == 0 else nc.scalar
            eng.dma_start(
                out=out_d[kc, :, s * WS + y, :].rearrange("c (w d) -> c w d", d=WS),
                in_=x2v[:, kc, :, y, :])


def _pin_act_table():
    """Pin all ScalarE activations to the one table containing every function
    this kernel uses (Copy/Exp/Ln/Square/Relu/MemsetZero). The default
    per-instruction greedy table pick alternates between the exp- and
    ln-tables on our Ln->Exp chains, costing an ACT_TABLE_LOAD (~1.5us) per
    switch. Table ids must stay positionally stable, so keep all entries and
    empty the sets of every other table."""
    if getattr(bacc, "_act_tables_pinned", False):
        return
    orig = bacc.get_activation_tables

    def pinned(arch):
        tables = dict(orig(arch))
        keep = "natural_log_exp_and_others"
        assert keep in tables
        return {name: (funcs if name == keep else set())
                for name, funcs in tables.items()}

    pinned.__wrapped__ = orig
    bacc.get_activation_tables = pinned
    bacc._act_tables_pinned = True


def _build(n_stripes):
    _pin_act_table()
    nc = bacc.Bacc("TRN2", target_bir_lowering=False, debug=False)

    x_d = nc.dram_tensor("x", [2, 128, H, W], F32, kind="ExternalInput")
    bf_d = nc.dram_tensor("bf", [N_STRIPES, 128, NPAIR], F32, kind="ExternalInput")
    wq_d = nc.dram_tensor("wq", [128, 2, C], F16, kind="ExternalInput")
    wk_d = nc.dram_tensor("wk", [128, 2, C], F16, kind="ExternalInput")
    wv_d = nc.dram_tensor("wv", [128, 2, C], F16, kind="ExternalInput")
    wp_d = nc.dram_tensor("wp", [128, 2, C], F16, kind="ExternalInput")
    wsp_d = nc.dram_tensor("wsp", [128, 2, 1], F16, kind="ExternalInput")
    w1_d = nc.dram_tensor("w1", [128, 2, FF], F16, kind="ExternalInput")
    w2_d = nc.dram_tensor("w2", [128, 8, C], F16, kind="ExternalInput")
    ws2_d = nc.dram_tensor("ws2", [128, 8, 1], F16, kind="ExternalInput")
    sel_d = nc.dram_tensor("sel", [128, 128], F16, kind="ExternalInput")
    n1g_d = nc.dram_tensor("n1g", [128, 2], F32, kind="ExternalInput")
    n1b_d = nc.dram_tensor("n1b", [128, 2], F32, kind="ExternalInput")
    n2g_d = nc.dram_tensor("n2g", [128, 2], F32, kind="ExternalInput")
    n2b_d = nc.dram_tensor("n2b", [128, 2], F32, kind="ExternalInput")
    out_d = nc.dram_tensor("out", [2, 128, H, W], F32, kind="ExternalOutput")
    dbg = None
    if os.environ.get("KERN_DEBUG", "0") == "1":
        dbg = {
            "q": nc.dram_tensor("dbg_q", [128, 2, TOK], F16, kind="ExternalOutput"),
            "k": nc.dram_tensor("dbg_k", [128, 2, TOK], F16, kind="ExternalOutput"),
            "v": nc.dram_tensor("dbg_v", [128, NPAIR, C], F16, kind="ExternalOutput"),
            "av": nc.dram_tensor("dbg_av", [128, 2, TOK], F16, kind="ExternalOutput"),
            "x1": nc.dram_tensor("dbg_x1", [128, 2, TOK], F32, kind="ExternalOutput"),
            "x1h": nc.dram_tensor("dbg_x1h", [128, 2, TOK], F16, kind="ExternalOutput"),
            "h": nc.dram_tensor("dbg_h", [128, 8, 512], F16, kind="ExternalOutput"),
        }

    with tile.TileContext(nc) as tc, ExitStack() as ctx:
        E = SimpleNamespace()
        for nm, bufs, space in (
                ("wpool", 1, "SBUF"), ("xin", 2, "SBUF"), ("qkvp", 2, "SBUF"),
                ("ep", 6, "SBUF"), ("avp", 2, "SBUF"), ("x1p", 2, "SBUF"),
                ("hp", 2, "SBUF"), ("sqp", 2, "SBUF"), ("rowp", 1, "SBUF"),
                ("abp", 2, "SBUF"), ("wtp", 2, "SBUF"), ("x2p", 2, "SBUF"),
                ("dbp", 3, "SBUF"), ("drp", 2, "DRAM"),
                ("ps_lin", 2, "PSUM"), ("ps_row", 1, "PSUM"),
                ("ps_sc", 1, "PSUM"), ("ps_av", 1, "PSUM")):
            setattr(E, nm, ctx.enter_context(
                tc.tile_pool(name=nm, bufs=bufs, space=space)))

        E.wq_s = E.wpool.tile([128, 2, C], F16)
        E.wk_s = E.wpool.tile([128, 2, C], F16)
        E.wv_s = E.wpool.tile([128, 2, C], F16)
        E.wp_s = E.wpool.tile([128, 2, C], F16)
        E.wsp_s = E.wpool.tile([128, 2, 1], F16)
        E.w1_s = E.wpool.tile([128, 2, FF], F16)
        E.w2_s = E.wpool.tile([128, 8, C], F16)
        E.ws2_s = E.wpool.tile([128, 8, 1], F16)
        E.n1g_s = E.wpool.tile([128, 2], F32)
        E.n1b_s = E.wpool.tile([128, 2], F32)
        E.n2g_s = E.wpool.tile([128, 2], F32)
        E.n2b_s = E.wpool.tile([128, 2], F32)
        E.sel_s = E.wpool.tile([128, 128], F16)
        E.ones_row = E.wpool.tile([1, 128], F16)
        E.ones2 = E.wpool.tile([128, 32], F16)   # col0: j in A, col1: j in B, rest 0
        E.eps_s = E.wpool.tile([128, 1], F32)
        E.ones_col = E.wpool.tile([128, 1], F16)
        E.blur_s = E.wpool.tile([128, N_STRIPES, NPAIR], F32)
        for dst, src in ((E.wq_s, wq_d), (E.wk_s, wk_d),
                         (E.wv_s, wv_d), (E.wp_s, wp_d), (E.wsp_s, wsp_d),
                         (E.w1_s, w1_d), (E.w2_s, w2_d), (E.ws2_s, ws2_d)):
            nc.sync.dma_start(out=dst, in_=src[:, :, :])
        nc.sync.dma_start(out=E.sel_s, in_=sel_d[:, :])
        for dst, src in ((E.n1g_s, n1g_d), (E.n1b_s, n1b_d),
                         (E.n2g_s, n2g_d), (E.n2b_s, n2b_d)):
            nc.sync.dma_start(out=dst, in_=src[:, :])
        nc.vector.memset(E.ones_row, 1.0)
        nc.vector.memset(E.eps_s, EPS)
        nc.vector.memset(E.ones2, 1.0)
        nc.vector.memset(E.ones2[64:128, 0:1], 0.0)
        nc.vector.memset(E.ones2[0:64, 1:2], 0.0)
        nc.vector.memset(E.ones_col, 1.0)
        nc.sync.dma_start(out=E.blur_s, in_=bf_d[:, :, :].rearrange("s p q -> p s q"))

        for s in range(n_stripes):
            _stripe(nc, E, s, x_d, out_d, dbg)

    nc.finalize()
    return nc


def _prep_weights(qkv_w, proj_w, ff1_w, ff2_w, n1_g, n1_b, n2_g, n2_b):
    wq = (qkv_w[:, 0:C] * SCALE).astype(np.float32)
    wk = qkv_w[:, C:2 * C].astype(np.float32)
    wv = qkv_w[:, 2 * C:3 * C].astype(np.float32)

    def fold(a, kchunks):
        cin, m = a.shape
        return np.ascontiguousarray(a.reshape(kchunks, 128, m).transpose(1, 0, 2))

    sel = np.zeros((128, 128), dtype=np.float16)
    for q in range(4):
        sel[32 * q, 0:64] = 1.0
        sel[32 * q + 1, 64:128] = 1.0

    return {
        "sel": sel,
        "wq": fold(wq.astype(np.float16), 2),
        "wk": fold(wk.astype(np.float16), 2),
        "wv": fold(wv.astype(np.float16), 2),
        "wp": fold(proj_w.astype(np.float16), 2),
        "wsp": fold(proj_w.sum(axis=1, keepdims=True).astype(np.float16), 2),
        "w1": fold(ff1_w.astype(np.float16), 2),
        "w2": fold(ff2_w.astype(np.float16), 8),
        "ws2": fold(ff2_w.sum(axis=1, keepdims=True).astype(np.float16), 8),
        "n1g": np.ascontiguousarray(n1_g.astype(np.float32).reshape(2, 128).T),
        "n1b": np.ascontiguousarray(n1_b.astype(np.float32).reshape(2, 128).T),
        "n2g": np.ascontiguousarray(n2_g.astype(np.float32).reshape(2, 128).T),
        "n2b": np.ascontiguousarray(n2_b.astype(np.float32).reshape(2, 128).T),
    }


def kernel(x, blur_map, qkv_w, qkv_b, proj_w, proj_b, ff1_w, ff1_b, ff2_w,
           ff2_b, n1_g, n1_b, n2_g, n2_b):
    assert np.abs(np.asarray(proj_b)).max() == 0.0, "requires proj_b == 0"
    assert np.abs(np.asarray(qkv_b)).max() == 0.0, "requires qkv_b == 0"
    assert np.abs(np.asarray(ff1_b)).max() == 0.0, "requires ff1_b == 0"
    assert np.abs(np.asarray(ff2_b)).max() == 0.0, "requires ff2_b == 0"

    n_stripes = int(os.environ.get("KERN_STRIPES", N_STRIPES))
    key = ("nc", n_stripes)
    if key not in _CACHED:
        _CACHED[key] = _build(n_stripes)
    nc = _CACHED[key]

    wdict = _prep_weights(np.asarray(qkv_w), np.asarray(proj_w),
                          np.asarray(ff1_w), np.asarray(ff2_w),
                          np.asarray(n1_g), np.asarray(n1_b),
                          np.asarray(n2_g), np.asarray(n2_b))

    blur_full = _bilinear_resize_x4(np.asarray(blur_map, dtype=np.float32))
    fac = 1.0 + BLUR_STRENGTH * blur_full[:, 0]                  # [B, H, W]
    fac = fac.reshape(B, N_STRIPES, WS, NW_X, WS)                # b, wy, dy, wx, dx
    fac = fac.transpose(0, 1, 3, 2, 4).reshape(B, N_STRIPES, NPAIR, 2 * T)
    fac = np.ascontiguousarray(fac.transpose(0, 1, 3, 2), dtype=np.float32)

    xs = np.asarray(x, dtype=np.float32).reshape(B, 2, 128, H, W)

    in_maps = []
    for b in range(B):
        m = dict(wdict)
        m["x"] = np.ascontiguousarray(xs[b])
        m["bf"] = fac[b]
        in_maps.append(m)

    _CACHED["last_run"] = (nc, in_maps)
    r = run_bass_kernel_spmd(nc, in_maps, list(range(8)))
    _CACHED["results"] = r.results
    out = np.stack([r.results[b]["out"].reshape(C, H, W) for b in range(B)])
    return out.astype(np.float32)


def run_traced(tmpdir=None):
    nc, in_maps = _CACHED["last_run"]
    return run_bass_kernel_spmd(nc, in_maps, list(range(8)), trace=True,
                                tmpdir=tmpdir)

